# revision 6
# baseline (speedup 1.0000x reference)
"""DeepSets segment-reduce kernel for 8x TRN2 NeuronCores (Bass/Tile).

Computes: out = rho_mlp(segment_mean(phi_mlp(ins), batch))  for
sorted segment ids `batch` in [0, 50000), ins [1M, 128] f32.

Strategy (see inline notes):
  - Segments are grouped in windows of 128. Windows are assigned
    round-robin-contiguously to the 8 cores (49 windows/core); every row
    of a window's segments goes to that window's core, so there is no
    cross-core reduction at all.
  - Host pre-transposes ins per window into fp16 [128, slots] tiles
    (padded to a uniform per-window slot count so a single SPMD NEFF
    serves all cores; pad rows have rinv=0 so they contribute nothing).
  - On device, per window: h1 = relu(X @ W1 + b1) via PE matmuls with
    the X^T block as the stationary operand (row-major output), b1 added
    with a K=2 rank-1 matmul (hi/lo fp16 split of b1).  The segment
    reduction is a matmul with a one-hot selection matrix S built on the
    DVE: S[r, s] = (rel_id[r] == s) * invcount[r], which folds the
    segment-mean directly into the reduction:  T = S^T @ h1r.
    The phi second layer is reassociated to run on segment means
    (20x fewer rows):  seg_mean = T @ W2 + b2*nz, followed by the rho
    MLP per window, all as [128,128] matmuls.
  - Everything intermediate is fp16 (PE runs fp16 at full rate and all
    values are O(1)); measured end-to-end scale-relative absmax vs the
    fp32 reference is ~4e-4.

kernel(**inputs) takes the full unsharded inputs and returns the full
[50000, 128] fp32 output.
"""

import numpy as np

import concourse.mybir as mybir
import concourse.tile as tile
from concourse import bacc
from concourse.bass_utils import run_bass_kernel_spmd

P = 128
N_CORES = 8
F16 = mybir.dt.float16
F32 = mybir.dt.float32


def _f16(a):
    return np.asarray(a, dtype=np.float32).astype(np.float16)


def _split16(a):
    hi = _f16(a)
    lo = _f16(np.asarray(a, dtype=np.float32) - hi.astype(np.float32))
    return hi, lo


def _host_prep(ins, batch, n_segs):
    """Shard rows by 128-segment windows; build per-core device arrays."""
    n = ins.shape[0]
    batch = np.asarray(batch).astype(np.int64)
    nwin_tot = -(-n_segs // P)                      # 391 for 50000
    nw = -(-nwin_tot // N_CORES)                    # windows per core (49)
    nwin_pad = nw * N_CORES                         # 392
    segs_pad = nwin_pad * P                         # 50176

    counts = np.bincount(batch, minlength=segs_pad).astype(np.float64)
    invc = np.where(counts > 0, 1.0 / np.maximum(counts, 1.0), 0.0)
    nz = (counts > 0).astype(np.float16)

    # window row ranges (batch is sorted)
    bounds = np.searchsorted(batch, np.arange(nwin_pad + 1) * P, side="left")
    win_cnt = np.diff(bounds)
    nb = max(1, int(-(-win_cnt.max() // P)))        # blocks of 128 rows per window
    slots = nb * P

    row_invc = invc[batch].astype(np.float32)       # [n]
    ins = np.asarray(ins, dtype=np.float32)

    per_core = []
    for c in range(N_CORES):
        xt = np.zeros((P, nw * slots), dtype=np.float16)
        rel = np.full((P, nw * nb), -1.0, dtype=np.float32)
        rinv = np.zeros((P, nw * nb), dtype=np.float32)
        for w in range(nw):
            g = c * nw + w
            s, e = bounds[g], bounds[g + 1]
            cnt = e - s
            if cnt == 0:
                continue
            xt[:, w * slots : w * slots + cnt] = ins[s:e].T
            relpad = np.full(slots, -1.0, dtype=np.float32)
            relpad[:cnt] = (batch[s:e] - g * P).astype(np.float32)
            rinvpad = np.zeros(slots, dtype=np.float32)
            rinvpad[:cnt] = row_invc[s:e]
            rel[:, w * nb : (w + 1) * nb] = relpad.reshape(nb, P).T
            rinv[:, w * nb : (w + 1) * nb] = rinvpad.reshape(nb, P).T
        nz2 = np.ascontiguousarray(
            np.broadcast_to(nz[c * nw * P : (c + 1) * nw * P], (2, nw * P))
        )
        per_core.append({"xt": xt, "rel": rel, "rinv": rinv, "nz2": nz2})
    return per_core, nw, nb


def _host_consts(wts, nw, nb):
    b1hi, b1lo = _split16(wts["phi_b1"])
    b2hi, b2lo = _split16(wts["phi_b2"])
    return {
        "w1": _f16(wts["phi_W1"]),                      # [128,128] lhs natural
        "w2": _f16(wts["phi_W2"]),
        "rw1": _f16(wts["rho_W1"]),
        "rw2": _f16(wts["rho_W2"]),
        "b1row": np.stack([np.tile(b1hi, 4), np.tile(b1lo, 4)]),   # [2, 512] f16
        "b2c2": np.stack([b2hi, b2lo]),                  # [2, 128] f16
        "rb1": np.asarray(wts["rho_b1"], np.float32).reshape(P, 1),
        "rb2": np.asarray(wts["rho_b2"], np.float32).reshape(P, 1),
        "iota": np.ascontiguousarray(
            np.broadcast_to(np.arange(P, dtype=np.float16), (P, P))
        ),
        "ident": np.eye(P, dtype=np.float16),
        "ones2": np.ones((2, P), dtype=np.float16),
    }


def _build(nw, nb, consts_np):
    """Emit the SPMD single-core program (same NEFF for all 8 cores)."""
    slots = nb * P
    nc = bacc.Bacc("TRN2", target_bir_lowering=False, debug=False,
                   num_devices=N_CORES)

    d_xt = nc.dram_tensor("xt", [P, nw * slots], F16, kind="ExternalInput").ap()
    d_rel = nc.dram_tensor("rel", [P, nw * nb], F32, kind="ExternalInput").ap()
    d_rinv = nc.dram_tensor("rinv", [P, nw * nb], F32, kind="ExternalInput").ap()
    d_nz2 = nc.dram_tensor("nz2", [2, nw * P], F16, kind="ExternalInput").ap()
    d_consts = {
        k: nc.dram_tensor(
            k, list(v.shape), mybir.dt.from_np(v.dtype), kind="ExternalInput"
        ).ap()
        for k, v in consts_np.items()
    }
    d_out = nc.dram_tensor("outT", [P, nw * P], F32, kind="ExternalOutput").ap()

    # chunking of a window's blocks into psum-bank sized pieces (<=4 blocks)
    chunks = []
    off = 0
    while off < nb:
        cs = min(4, nb - off)
        chunks.append((off, cs))
        off += cs

    with tile.TileContext(nc) as tc:
        with (
            tc.tile_pool(name="const", bufs=1) as constp,
            tc.tile_pool(name="outsb", bufs=1) as outp,
            tc.tile_pool(name="xt", bufs=3) as xtp,
            tc.tile_pool(name="h1r", bufs=2) as h1rp,
            tc.tile_pool(name="s", bufs=4) as sp,
            tc.tile_pool(name="tail16", bufs=6) as tailp,
            tc.tile_pool(name="h1ps", bufs=2, space="PSUM") as h1psp,
            tc.tile_pool(name="tps", bufs=2, space="PSUM") as tpsp,
            tc.tile_pool(name="tailps", bufs=2, space="PSUM") as tailpsp,
        ):
            cs_ = {}
            for k, v in consts_np.items():
                cs_[k] = constp.tile(
                    list(v.shape), mybir.dt.from_np(v.dtype), name=f"c_{k}"
                )
                nc.sync.dma_start(cs_[k], d_consts[k])
            relsb = constp.tile([P, nw * nb], F32)
            nc.sync.dma_start(relsb, d_rel)
            rinvsb = constp.tile([P, nw * nb], F32)
            nc.sync.dma_start(rinvsb, d_rinv)
            nz2sb = constp.tile([2, nw * P], F16)
            nc.sync.dma_start(nz2sb, d_nz2)
            outsb = outp.tile([P, nw * P], F32)

            for w in range(nw):
                xt = xtp.tile([P, slots], F16)
                nc.sync.dma_start(xt, d_xt[:, w * slots : (w + 1) * slots])

                # ---- phi layer 1: h1r = relu(X @ W1 + b1), row-major out
                h1r = h1rp.tile([P, slots], F16)
                for coff, csz in chunks:
                    h1ps = h1psp.tile([P, 512], F32, space="PSUM", tag="h1ps")
                    reg = h1ps[:, : csz * P]
                    # b1 (hi+lo fp16 rows) broadcast to all rows: K=2 rank-1
                    nc.tensor.matmul(
                        reg, lhsT=cs_["ones2"], rhs=cs_["b1row"][:, : csz * P],
                        start=True, stop=False,
                    )
                    for j in range(csz):
                        b = coff + j
                        nc.tensor.matmul(
                            h1ps[:, j * P : (j + 1) * P],
                            lhsT=xt[:, b * P : (b + 1) * P],
                            rhs=cs_["w1"],
                            start=False, stop=(j == csz - 1),
                        )
                    nc.scalar.activation(
                        h1r[:, coff * P : (coff + csz) * P], reg,
                        mybir.ActivationFunctionType.Relu,
                    )

                # ---- segment-mean reduction (one-hot matmul), fused invcount
                tps = tpsp.tile([P, P], F32, space="PSUM", tag="tps")
                for b in range(nb):
                    col = w * nb + b
                    s_t = sp.tile([P, P], F16)
                    nc.vector.tensor_scalar(
                        s_t, cs_["iota"],
                        relsb[:, col : col + 1], rinvsb[:, col : col + 1],
                        op0=mybir.AluOpType.is_equal, op1=mybir.AluOpType.mult,
                    )
                    nc.tensor.matmul(
                        tps, lhsT=s_t, rhs=h1r[:, b * P : (b + 1) * P],
                        start=(b == 0), stop=(b == nb - 1),
                    )

                # ---- window tail: T -> transpose -> W2/b2 -> rho MLP
                t_sb = tailp.tile([P, P], F16, tag="t_sb")
                nc.scalar.copy(t_sb, tps)
                trps = tailpsp.tile([P, P], F16, space="PSUM", tag="trps", bufs=1)
                nc.tensor.transpose(trps, t_sb, cs_["ident"])
                tp_sb = tailp.tile([P, P], F16, tag="tp_sb")
                nc.scalar.copy(tp_sb, trps)

                smps = tailpsp.tile([P, P], F32, space="PSUM", tag="tailps")
                nc.tensor.matmul(smps, lhsT=cs_["w2"], rhs=tp_sb,
                                 start=True, stop=False)
                nc.tensor.matmul(
                    smps, lhsT=cs_["b2c2"], rhs=nz2sb[:, w * P : (w + 1) * P],
                    start=False, stop=True,
                )
                sm_sb = tailp.tile([P, P], F16, tag="sm_sb")
                nc.scalar.copy(sm_sb, smps)

                r1ps = tailpsp.tile([P, P], F32, space="PSUM", tag="tailps")
                nc.tensor.matmul(r1ps, lhsT=cs_["rw1"], rhs=sm_sb,
                                 start=True, stop=True)
                r1_sb = tailp.tile([P, P], F16, tag="r1_sb")
                nc.scalar.activation(
                    r1_sb, r1ps, mybir.ActivationFunctionType.Relu,
                    bias=cs_["rb1"][:, :1],
                )

                ops_ = tailpsp.tile([P, P], F32, space="PSUM", tag="tailps")
                nc.tensor.matmul(ops_, lhsT=cs_["rw2"], rhs=r1_sb,
                                 start=True, stop=True)
                nc.scalar.activation(
                    outsb[:, w * P : (w + 1) * P], ops_,
                    mybir.ActivationFunctionType.Identity,
                    bias=cs_["rb2"][:, :1],
                )

            nc.sync.dma_start(d_out, outsb)

    nc.compile()
    return nc


def _run(inputs, n_segs=50000, trace=False, **hw_kwargs):
    ins = np.asarray(inputs["ins"])
    batch = np.asarray(inputs["batch"])
    per_core, nw, nb = _host_prep(ins, batch, n_segs)
    consts_np = _host_consts(inputs, nw, nb)
    nc = _build(nw, nb, consts_np)

    in_maps = []
    for c in range(N_CORES):
        m = dict(consts_np)
        m.update(per_core[c])
        in_maps.append(m)
    res = run_bass_kernel_spmd(
        nc, in_maps, core_ids=list(range(N_CORES)), trace=trace, **hw_kwargs
    )
    outs = [r["outT"] for r in res.results]           # each [128, nw*128] f32
    full = np.concatenate([o.T for o in outs], axis=0)  # [8*nw*128, 128]
    return np.ascontiguousarray(full[:n_segs]), res


def kernel(**inputs):
    out, _ = _run(inputs)
    return out


# revision 7
# speedup vs baseline: 1.1985x; 1.1985x over previous
"""DeepSets segment-reduce kernel for 8x TRN2 NeuronCores (Bass/Tile).

Computes: out = rho_mlp(segment_mean(phi_mlp(ins), batch))  for
sorted segment ids `batch` in [0, 50000), ins [1M, 128] f32.

Strategy:
  - Segments are grouped in windows of 128, windows assigned contiguously
    to the 8 cores (no segment straddles a core => zero cross-core
    collectives). One SPMD NEFF serves all cores; per-window row counts
    are padded to a uniform block count (pad rows contribute exactly 0).
  - Host preprocessing (numpy): transpose ins per window into fp16
    [128, slots] tiles with each row pre-scaled by 1/count(segment) (this
    folds the segment-mean into the phi layer-1 output, and zeroes pad
    rows), plus fp8 one-hot selection matrices S per 128-row block.
  - Device, per window:
      h1 = relu(Xs @ W1 + rinv*b1)   -- X^T block stationary, W1 moving;
                                        bias via K=4 block-diag rank-1
      T' = sum_b h1r_b^T-contracted  -- matmul lhsT=h1r_b, rhs=S_b (fp8)
           => T'[hid, seg] accumulated in PSUM = segment-MEANS of h1r
      seg_mean^T = W2^T @ T' + b2 x nz  (phi layer 2 reassociated onto
           50k segments instead of 1M rows, ~20x less work)
      rho MLP on seg_mean^T per window ([128,128] matmuls), biases via
      per-partition ACT bias in the transposed orientation.
  - All intermediates fp16 (full PE rate, values O(1)); end-to-end
    scale-relative absmax vs fp32 reference ~5e-4.

kernel(**inputs) takes the full unsharded inputs and returns the full
[50000, 128] fp32 output.
"""

import numpy as np
import ml_dtypes

import concourse.mybir as mybir
import concourse.tile as tile
from concourse import bacc
from concourse.bass_utils import run_bass_kernel_spmd

P = 128
N_CORES = 8
F16 = mybir.dt.float16
F32 = mybir.dt.float32
F8 = mybir.dt.float8e4
FP8NP = ml_dtypes.float8_e4m3


def _f16(a):
    return np.asarray(a, dtype=np.float32).astype(np.float16)


def _host_prep(ins, batch, wts, n_segs):
    """Shard rows by 128-segment windows; build per-core device arrays."""
    batch = np.asarray(batch).astype(np.int64)
    nwin_tot = -(-n_segs // P)                      # 391 for 50000
    nw = -(-nwin_tot // N_CORES)                    # windows per core (49)
    nwin_pad = nw * N_CORES                         # 392
    segs_pad = nwin_pad * P                         # 50176

    counts = np.bincount(batch, minlength=segs_pad).astype(np.float64)
    invc = np.where(counts > 0, 1.0 / np.maximum(counts, 1.0), 0.0)
    nz = (counts > 0).astype(np.float16)

    bounds = np.searchsorted(batch, np.arange(nwin_pad + 1) * P, side="left")
    win_cnt = np.diff(bounds)
    nb = max(1, int(-(-win_cnt.max() // P)))        # 128-row blocks per window
    slots = nb * P
    nch = -(-nb // 4)                               # psum chunks per window

    row_invc = invc[batch].astype(np.float32)
    ins = np.asarray(ins, dtype=np.float32)
    seg_ar = np.arange(P, dtype=np.int64)

    per_core = []
    for c in range(N_CORES):
        xt = np.zeros((P, nw * slots), dtype=np.float16)
        relpad = np.full((nw, slots), -1, dtype=np.int64)
        rinvpad = np.zeros((nw, slots), dtype=np.float32)
        for w in range(nw):
            g = c * nw + w
            s, e = bounds[g], bounds[g + 1]
            cnt = e - s
            if cnt == 0:
                continue
            # fp16 X^T, pre-scaled by 1/count so the segment reduction
            # directly produces means (pad rows scale to exactly 0)
            xt[:, w * slots : w * slots + cnt] = \
                (ins[s:e] * row_invc[s:e, None]).T
            relpad[w, :cnt] = batch[s:e] - g * P
            rinvpad[w, :cnt] = row_invc[s:e]
        # fp8 one-hot S per block: S[r, s] = (rel[r] == s)
        oneh = (relpad.reshape(nw * nb, P)[:, :, None] == seg_ar)  # [blk,r,s]
        sfp8 = np.ascontiguousarray(
            oneh.transpose(1, 0, 2).reshape(P, nw * nb * P)
        ).astype(FP8NP)
        # K=4 rank-1 bias weights: rinv rows per chunk-block
        rinv4 = np.zeros((4, nw * nch * P), dtype=np.float16)
        r3 = rinvpad.reshape(nw, nb, P)
        for ci in range(nch):
            csz = min(4, nb - ci * 4)
            for k in range(csz):
                col = (np.arange(nw) * nch + ci)[:, None] * P + np.arange(P)
                rinv4[k, col.ravel()] = r3[:, ci * 4 + k, :].astype(
                    np.float16).ravel()
        nz2 = np.ascontiguousarray(np.broadcast_to(
            nz[c * nw * P : (c + 1) * nw * P], (2, nw * P)))
        per_core.append({"xt": xt, "sfp8": sfp8, "rinv4": rinv4, "nz2": nz2})
    return per_core, nw, nb


def _host_consts(wts):
    b1 = _f16(wts["phi_b1"])
    b1diag = np.zeros((4, 512), dtype=np.float16)
    for k in range(4):
        b1diag[k, k * P : (k + 1) * P] = b1
    b2hi = _f16(wts["phi_b2"])
    b2lo = _f16(np.asarray(wts["phi_b2"], np.float32) - b2hi.astype(np.float32))
    return {
        "w1": _f16(wts["phi_W1"]),
        "w2": _f16(wts["phi_W2"]),
        "rw1": _f16(wts["rho_W1"]),
        "rw2": _f16(wts["rho_W2"]),
        "b1diag": b1diag,
        "b2c2": np.stack([b2hi, b2lo]),
        "rb1": np.asarray(wts["rho_b1"], np.float32).reshape(P, 1),
        "rb2": np.asarray(wts["rho_b2"], np.float32).reshape(P, 1),
    }


def _build(nw, nb, consts_np):
    """Emit the SPMD single-core program (same NEFF for all 8 cores)."""
    slots = nb * P
    nch = -(-nb // 4)
    nc = bacc.Bacc("TRN2", target_bir_lowering=False, debug=False,
                   num_devices=N_CORES)

    d_xt = nc.dram_tensor("xt", [P, nw * slots], F16, kind="ExternalInput").ap()
    d_s = nc.dram_tensor("sfp8", [P, nw * nb * P], F8,
                         kind="ExternalInput").ap()
    d_r4 = nc.dram_tensor("rinv4", [4, nw * nch * P], F16,
                          kind="ExternalInput").ap()
    d_nz2 = nc.dram_tensor("nz2", [2, nw * P], F16, kind="ExternalInput").ap()
    d_consts = {
        k: nc.dram_tensor(
            k, list(v.shape), mybir.dt.from_np(v.dtype), kind="ExternalInput"
        ).ap()
        for k, v in consts_np.items()
    }
    d_out = nc.dram_tensor("outT", [P, nw * P], F32, kind="ExternalOutput").ap()

    chunks = []
    off = 0
    while off < nb:
        cs = min(4, nb - off)
        chunks.append((off, cs))
        off += cs

    with tile.TileContext(nc) as tc:
        with (
            tc.tile_pool(name="const", bufs=1) as constp,
            tc.tile_pool(name="outsb", bufs=1) as outp,
            tc.tile_pool(name="xt", bufs=3) as xtp,
            tc.tile_pool(name="sfp", bufs=3) as sfpp,
            tc.tile_pool(name="h1r", bufs=2) as h1rp,
            tc.tile_pool(name="tail16", bufs=6) as tailp,
            tc.tile_pool(name="h1ps", bufs=3, space="PSUM") as h1psp,
            tc.tile_pool(name="tps", bufs=2, space="PSUM") as tpsp,
            tc.tile_pool(name="tailps", bufs=2, space="PSUM") as tailpsp,
        ):
            cs_ = {}
            for k, v in consts_np.items():
                cs_[k] = constp.tile(
                    list(v.shape), mybir.dt.from_np(v.dtype), name=f"c_{k}"
                )
                nc.sync.dma_start(cs_[k], d_consts[k])
            r4sb = constp.tile([4, nw * nch * P], F16)
            nc.sync.dma_start(r4sb, d_r4)
            nz2sb = constp.tile([2, nw * P], F16)
            nc.sync.dma_start(nz2sb, d_nz2)
            outsb = outp.tile([P, nw * P], F32)

            for w in range(nw):
                xt = xtp.tile([P, slots], F16)
                nc.sync.dma_start(xt, d_xt[:, w * slots : (w + 1) * slots])
                st = sfpp.tile([P, nb * P], F8)
                nc.sync.dma_start(
                    st, d_s[:, w * nb * P : (w + 1) * nb * P])

                # ---- phi layer 1: h1r = relu(Xs @ W1 + rinv*b1), row-major
                h1r = h1rp.tile([P, slots], F16)
                for ci, (coff, csz) in enumerate(chunks):
                    h1ps = h1psp.tile([P, 512], F32, space="PSUM", tag="h1ps")
                    reg = h1ps[:, : csz * P]
                    c4 = (w * nch + ci) * P
                    nc.tensor.matmul(
                        reg, lhsT=r4sb[:, c4 : c4 + P],
                        rhs=cs_["b1diag"][:, : csz * P],
                        start=True, stop=False,
                    )
                    for j in range(csz):
                        b = coff + j
                        nc.tensor.matmul(
                            h1ps[:, j * P : (j + 1) * P],
                            lhsT=xt[:, b * P : (b + 1) * P],
                            rhs=cs_["w1"],
                            start=False, stop=(j == csz - 1),
                        )
                    dst = h1r[:, coff * P : (coff + csz) * P]
                    if ci % 2 == 0:
                        nc.scalar.activation(
                            dst, reg, mybir.ActivationFunctionType.Relu)
                    else:
                        nc.vector.tensor_scalar(
                            dst, reg, 0.0, None, op0=mybir.AluOpType.max)

                # ---- segment-mean reduction: T'[hid, seg] += h1r_b^T @ S_b
                tps = tpsp.tile([P, P], F32, space="PSUM", tag="tps")
                for b in range(nb):
                    nc.tensor.matmul(
                        tps, lhsT=h1r[:, b * P : (b + 1) * P],
                        rhs=st[:, b * P : (b + 1) * P],
                        start=(b == 0), stop=(b == nb - 1),
                    )
                t_sb = tailp.tile([P, P], F16, tag="t_sb")
                nc.scalar.copy(t_sb, tps)

                # ---- phi layer 2 on segment means: sm^T = W2^T@T' + b2 x nz
                smps = tailpsp.tile([P, P], F32, space="PSUM", tag="tailps")
                nc.tensor.matmul(smps, lhsT=cs_["w2"], rhs=t_sb,
                                 start=True, stop=False)
                nc.tensor.matmul(
                    smps, lhsT=cs_["b2c2"], rhs=nz2sb[:, w * P : (w + 1) * P],
                    start=False, stop=True,
                )
                sm_sb = tailp.tile([P, P], F16, tag="sm_sb")
                nc.vector.tensor_copy(sm_sb, smps)

                # ---- rho MLP (feature-major: per-partition ACT biases)
                r1ps = tailpsp.tile([P, P], F32, space="PSUM", tag="tailps")
                nc.tensor.matmul(r1ps, lhsT=cs_["rw1"], rhs=sm_sb,
                                 start=True, stop=True)
                r1_sb = tailp.tile([P, P], F16, tag="r1_sb")
                nc.scalar.activation(
                    r1_sb, r1ps, mybir.ActivationFunctionType.Relu,
                    bias=cs_["rb1"][:, :1],
                )
                ops_ = tailpsp.tile([P, P], F32, space="PSUM", tag="tailps")
                nc.tensor.matmul(ops_, lhsT=cs_["rw2"], rhs=r1_sb,
                                 start=True, stop=True)
                nc.scalar.activation(
                    outsb[:, w * P : (w + 1) * P], ops_,
                    mybir.ActivationFunctionType.Identity,
                    bias=cs_["rb2"][:, :1],
                )

            nc.sync.dma_start(d_out, outsb)

    nc.compile()
    return nc


def _run(inputs, n_segs=50000, trace=False, **hw_kwargs):
    ins = np.asarray(inputs["ins"])
    batch = np.asarray(inputs["batch"])
    per_core, nw, nb = _host_prep(ins, batch, inputs, n_segs)
    consts_np = _host_consts(inputs)
    nc = _build(nw, nb, consts_np)

    in_maps = []
    for c in range(N_CORES):
        m = dict(consts_np)
        m.update(per_core[c])
        in_maps.append(m)
    res = run_bass_kernel_spmd(
        nc, in_maps, core_ids=list(range(N_CORES)), trace=trace, **hw_kwargs
    )
    outs = [r["outT"] for r in res.results]             # [128, nw*128] f32
    full = np.concatenate([o.T for o in outs], axis=0)  # [8*nw*128, 128]
    return np.ascontiguousarray(full[:n_segs]), res


def kernel(**inputs):
    out, _ = _run(inputs)
    return out
